# revision 1
# baseline (speedup 1.0000x reference)
"""Trainium2 Bass kernel for CategoricalDistInstance (softmax pdf/log_prob/entropy).

Computes, for logits [B, V] and integer value [B]:
    probs   = softmax(logits, axis=-1)
    pdf     = probs[i, value[i]]                       # [B]
    log_prob= log(pdf)                                 # [B]
    entropy = sum(probs * log(probs), axis=-1)         # [B] (negative entropy)
    out     = stack([pdf, log_prob, entropy])          # [3, B]

Math used on-device (single pass over the data, no max subtraction —
logits are N(0,1) so exp() cannot overflow fp32):
    Z  = sum_c exp(x_c)          (per row)
    S  = sum_c x_c * exp(x_c)    (per row)
    pdf      = exp(x_v) / Z
    log_prob = x_v - log(Z)
    entropy  = S/Z - log(Z)

Sharding: pure data-parallel over the batch dim across 8 NeuronCores
(512 rows each). No communication.

Per-core pipeline (rows-on-partitions, 4 row blocks of 128 x 8 col
chunks of 4000; the final chunks taper to shorten the post-DMA tail):
    DMA   : load chunk x [128, 4000] (2 MB per DMA)       (HWDGE on SP)
    ACT   : u = exp(x), fused accum_out -> Z partial      (1 pass)
    DVE   : custom TENSOR_TENSOR_REDUCE u*x -> S partial  (1 pass,
            stride-0 dummy out, only the accumulator is kept)
The x_v gather (indirect DMA via SWDGE) runs in a prologue; all Ln work
is batched into one op at the end (single ACT table switch); one
[128, 4, 3] result store (ACT HWDGE ring) finishes the kernel.

Measured (axon TRN2, 8 cores): ~200 us/iteration steady state,
~330 GB/s/core of HBM read — ~92% of the 358 GB/s HBM-per-NC bound.
TimelineSim single-shot estimate: ~196 us.
"""

import numpy as np

import concourse.bacc as bacc
import concourse.bass as bass
import concourse.mybir as mybir
import concourse.tile as tile
from concourse.bass_utils import run_bass_kernel_spmd
from concourse.dve_ops import TENSOR_TENSOR_REDUCE
from concourse.tile import add_dep_helper

B, V = 4096, 32000
NCORES = 8
R = B // NCORES  # 512 rows per core
P = 128          # SBUF partitions
NB = R // P      # 4 row blocks per core
CC = 4000        # column chunk size
NCH = V // CC    # 8 chunks per row block

X_BUFS = 6
U_BUFS = 3

_CACHE: dict = {}

# test.py can set this to request a profiled run
TRACE = False
LAST_RESULT = None


def _build_bass(reps: int = 1):
    """Build the per-core Bass program. reps>1 repeats the whole computation
    (for wall-clock benchmarking only)."""
    f32 = mybir.dt.float32
    i32 = mybir.dt.int32
    Exp = mybir.ActivationFunctionType.Exp
    Ln = mybir.ActivationFunctionType.Ln
    add = mybir.AluOpType.add
    mult = mybir.AluOpType.mult
    sub = mybir.AluOpType.subtract
    X = mybir.AxisListType.X

    nc = bacc.Bacc("TRN2", target_bir_lowering=False, debug=False)
    logits = nc.dram_tensor("logits", [R, V], f32, kind="ExternalInput")
    value = nc.dram_tensor("value", [R, 1], i32, kind="ExternalInput")
    out = nc.dram_tensor("out", [R, 3], f32, kind="ExternalOutput")

    # flat [R*V, 1] view for the per-row element gather
    logits_flat = logits.ap().rearrange("r (v o) -> (r v) o", o=1)
    # [P, NB] view of value: column b holds rows [b*P, (b+1)*P)
    value_pb = value.ap().rearrange("(b p) o -> p (b o)", p=P)
    # [P, NB, 3] view of out: (partition, block, result-col)
    out_pb = out.ap().rearrange("(b p) k -> p b k", p=P)

    with tile.TileContext(nc) as tc:
        with (
            tc.tile_pool(name="x", bufs=X_BUFS) as xp,
            tc.tile_pool(name="u", bufs=U_BUFS) as up,
            tc.tile_pool(name="small", bufs=2) as sp,
            tc.tile_pool(name="persist", bufs=1) as pers,
        ):
            for rep in range(reps):
                # --- gather prologue: everything that depends only on `value`
                # runs before the big streaming loop so its tiny DMAs don't
                # queue behind (or ahead of) the 2 MB chunk loads.
                # off[p, b] = (b*P + p)*V + value[b*P + p]
                vals = pers.tile([P, NB], i32, tag="vals")
                # SWDGE (Pool) so SP's HWDGE ring only ever issues the big
                # streaming chunk loads
                nc.gpsimd.dma_start(out=vals[:], in_=value_pb)
                ioff = pers.tile([P, NB], i32, tag="ioff")
                # one iota per column: the ISA caps free-dim iota steps at
                # int16, so b*P*V must go in via `base` instead of a step
                for rb in range(NB):
                    nc.gpsimd.iota(
                        ioff[:, rb : rb + 1],
                        pattern=[[0, 1]],
                        base=rb * P * V,
                        channel_multiplier=V,
                    )
                off = pers.tile([P, NB], i32, tag="off")
                nc.vector.tensor_tensor(out=off[:], in0=ioff[:], in1=vals[:], op=add)
                xvs = pers.tile([P, NB], f32, tag="xvs")
                for rb in range(NB):
                    nc.gpsimd.indirect_dma_start(
                        out=xvs[:, rb : rb + 1],
                        out_offset=None,
                        in_=logits_flat,
                        in_offset=bass.IndirectOffsetOnAxis(
                            ap=off[:, rb : rb + 1], axis=0
                        ),
                    )
                exvs = pers.tile([P, NB], f32, tag="exvs")
                exvs_inst = None
                # per-block row sums, finalized once at the end (keeps all Ln
                # work in one op -> one ACT table switch instead of per-block
                # exp<->ln ping-pong)
                Zall = pers.tile([P, NB], f32, tag="Zall")
                Sall = pers.tile([P, NB], f32, tag="Sall")

                for rb in range(NB):
                    rows = slice(rb * P, (rb + 1) * P)
                    # Taper the very last chunks of the run so the trailing
                    # ACT+DVE work after the final DMA lands is short.
                    if rb == NB - 1 and rep == reps - 1:
                        widths = [CC] * (NCH - 2) + [
                            CC // 2, CC // 2, CC // 2, CC // 4, CC // 8, CC // 8,
                        ]
                    else:
                        widths = [CC] * NCH
                    nch = len(widths)
                    zparts = sp.tile([P, nch], f32, tag="zparts")
                    sparts = sp.tile([P, nch], f32, tag="sparts")
                    last_exp_inst = None
                    c0 = 0
                    for ch, w in enumerate(widths):
                        cols = slice(c0, c0 + w)
                        c0 += w
                        x = xp.tile([P, w], f32, tag="x")
                        nc.sync.dma_start(out=x[:], in_=logits[rows, cols])
                        u = up.tile([P, w], f32, tag="u")
                        last_exp_inst = nc.scalar.activation(
                            u[:], x[:], Exp, accum_out=zparts[:, ch : ch + 1]
                        )
                        # fused multiply+reduce; stride-0 dummy out discards
                        # the product, only sparts[:, ch] (the sum) is kept
                        dummy = sp.tile([P, 1], f32, tag="dummy")
                        nc.vector._custom_dve(
                            TENSOR_TENSOR_REDUCE,
                            out=dummy.broadcast_to(u[:].shape),
                            in0=u[:],
                            in1=x[:],
                            s0=0.0,
                            s1=1.0,
                            accum_out=sparts[:, ch : ch + 1],
                        )

                    if rb == 0:
                        # exp of the gathered logits, for pdf. Created here (and
                        # ordered after block 0's chunk exps) so the scheduler
                        # cannot hoist it to the front of the ACT queue, where
                        # its wait on the tiny gather DMAs would head-of-line
                        # block the streaming exps.
                        exvs_inst = nc.scalar.activation(exvs[:], xvs[:], Exp)
                        add_dep_helper(
                            exvs_inst.ins,
                            last_exp_inst.ins,
                            sync=False,
                            reason="keep gather-exp behind block0 streaming exps",
                        )

                    # --- per-block partial reduction (DVE only, no ACT) ---
                    nc.vector.tensor_reduce(
                        Zall[:, rb : rb + 1], zparts[:], axis=X, op=add
                    )
                    nc.vector.tensor_reduce(
                        Sall[:, rb : rb + 1], sparts[:], axis=X, op=add
                    )

                # --- final epilogue, all blocks at once ([P, NB] ops) ---
                res = pers.tile([P, NB * 3], f32, tag="res")
                res3 = res[:].rearrange("p (b k) -> p b k", b=NB)
                rZ = pers.tile([P, NB], f32, tag="rZ")
                nc.vector.reciprocal(rZ[:], Zall[:])
                # pdf = exp(x_v) / Z  (independent of Ln -> can run during the
                # ACT table switch)
                nc.vector.tensor_mul(out=res3[:, :, 0], in0=exvs[:], in1=rZ[:])

                logZ = pers.tile([P, NB], f32, tag="logZ")
                nc.scalar.activation(logZ[:], Zall[:], Ln)
                # log_prob = x_v - log Z
                nc.vector.tensor_sub(out=res3[:, :, 1], in0=xvs[:], in1=logZ[:])
                # entropy = S/Z - log Z
                sz = pers.tile([P, NB], f32, tag="sz")
                nc.vector.tensor_mul(out=sz[:], in0=Sall[:], in1=rZ[:])
                nc.vector.tensor_sub(out=res3[:, :, 2], in0=sz[:], in1=logZ[:])
                # issue the store from ACT's HWDGE ring: an SP-issued store
                # would wait here for the epilogue and head-of-line block the
                # next rep's chunk loads queued behind it on SP
                nc.scalar.dma_start(out=out_pb, in_=res3)
    nc.compile()
    return nc


def kernel(logits, value):
    global LAST_RESULT
    logits = np.ascontiguousarray(np.asarray(logits), dtype=np.float32)
    value = np.asarray(value).astype(np.int32).reshape(B, 1)
    assert logits.shape == (B, V)

    if "nc" not in _CACHE:
        _CACHE["nc"] = _build_bass()
    nc = _CACHE["nc"]

    in_maps = [
        {
            "logits": np.ascontiguousarray(logits[c * R : (c + 1) * R]),
            "value": np.ascontiguousarray(value[c * R : (c + 1) * R]),
        }
        for c in range(NCORES)
    ]
    result = run_bass_kernel_spmd(
        nc, in_maps, core_ids=list(range(NCORES)), trace=TRACE
    )
    LAST_RESULT = result
    # each core's out is [R, 3]; full output is [3, B]
    full = np.concatenate([r["out"] for r in result.results], axis=0)  # [B, 3]
    return np.ascontiguousarray(full.T)



# revision 6
# speedup vs baseline: 178.9177x; 178.9177x over previous
"""Trainium2 Bass kernel for CategoricalDistInstance (softmax pdf/log_prob/entropy).

Computes, for logits [B, V] and integer value [B]:
    probs   = softmax(logits, axis=-1)
    pdf     = probs[i, value[i]]                       # [B]
    log_prob= log(pdf)                                 # [B]
    entropy = sum(probs * log(probs), axis=-1)         # [B] (negative entropy)
    out     = stack([pdf, log_prob, entropy])          # [3, B]

Math used on-device (single pass over the data, no max subtraction —
logits are N(0,1) so exp() cannot overflow fp32):
    Z  = sum_c exp(x_c)          (per row)
    S  = sum_c x_c * exp(x_c)    (per row)
    log_prob = x_v - log(Z)
    pdf      = exp(log_prob)
    entropy  = S/Z - log(Z)

Sharding: pure data-parallel over the batch dim across 8 NeuronCores
(512 rows each). No communication.

Per-core pipeline (rows-on-partitions, 4 row blocks of 128 x 8 col
chunks of 4000; the final chunks taper to shorten the post-DMA tail):
    DMA   : load chunk x [128, 4000] (2 MB per DMA)       (HWDGE on SP)
    ACT   : u = exp(x), fused accum_out -> Z partial      (1 pass)
    DVE   : custom TENSOR_TENSOR_REDUCE u*x -> S partial  (1 pass,
            stride-0 dummy out, only the accumulator is kept)
The x_v gather (indirect DMA via SWDGE) runs entirely on the Pool
engine (iota + offset add + gathers) so neither ACT nor DVE ever waits
on it mid-stream; all gathered-value math happens in the epilogue.
A single pre-placed LoadActFuncSet of the combined Exp+Ln table at
program start means ACT never reloads activation tables (the default
placement ping-pongs Exp-table <-> Ln-table once per iteration).
"""

import numpy as np

import concourse.bacc as bacc
import concourse.bass as bass
import concourse.mybir as mybir
import concourse.tile as tile
from concourse.bass_utils import run_bass_kernel_spmd
from concourse.dve_ops import TENSOR_TENSOR_REDUCE
from concourse.hw_specs import get_activation_tables

B, V = 4096, 32000
NCORES = 8
R = B // NCORES  # 512 rows per core
P = 128          # SBUF partitions
NB = R // P      # 4 row blocks per core
CC = 4000        # column chunk size
NCH = V // CC    # 8 chunks per row block

X_BUFS = 6
U_BUFS = 3

_CACHE: dict = {}

# test.py can set this to request a profiled run
TRACE = False
LAST_RESULT = None


def _build_bass(reps: int = 1):
    """Build the per-core Bass program. reps>1 repeats the whole computation
    (for wall-clock benchmarking only)."""
    f32 = mybir.dt.float32
    i32 = mybir.dt.int32
    Exp = mybir.ActivationFunctionType.Exp
    Ln = mybir.ActivationFunctionType.Ln
    add = mybir.AluOpType.add
    sub = mybir.AluOpType.subtract
    X = mybir.AxisListType.X

    nc = bacc.Bacc("TRN2", target_bir_lowering=False, debug=False)
    logits = nc.dram_tensor("logits", [R, V], f32, kind="ExternalInput")
    value = nc.dram_tensor("value", [R, 1], i32, kind="ExternalInput")
    out = nc.dram_tensor("out", [R, 3], f32, kind="ExternalOutput")

    # flat [R*V, 1] view for the per-row element gather
    logits_flat = logits.ap().rearrange("r (v o) -> (r v) o", o=1)
    # [P, NB] view of value: column b holds rows [b*P, (b+1)*P)
    value_pb = value.ap().rearrange("(b p) o -> p (b o)", p=P)
    # [P, NB, 3] view of out: (partition, block, result-col)
    out_pb = out.ap().rearrange("(b p) k -> p b k", p=P)

    with tile.TileContext(nc) as tc:
        with (
            tc.tile_pool(name="x", bufs=X_BUFS) as xp,
            tc.tile_pool(name="u", bufs=U_BUFS) as up,
            tc.tile_pool(name="small", bufs=2) as sp,
            tc.tile_pool(name="gat", bufs=2) as gp,
            tc.tile_pool(name="persist", bufs=1) as pers,
        ):
            # ioff[p, b] = (b*P + p)*V is iteration-invariant: compute once.
            # The ISA caps free-dim iota steps at int16, so b*P*V goes in via
            # `base` per column instead of a step.
            ioff = pers.tile([P, NB], i32, tag="ioff")
            for rb in range(NB):
                nc.gpsimd.iota(
                    ioff[:, rb : rb + 1],
                    pattern=[[0, 1]],
                    base=rb * P * V,
                    channel_multiplier=V,
                )
            for rep in range(reps):
                # --- gather prologue, entirely on Pool/SWDGE: nothing in the
                # streaming loop (SP DMA / ACT / DVE) ever waits on it; the
                # epilogue is its only consumer. (The 512 scattered 4B reads
                # cost ~8% of streaming bandwidth while in flight — measured;
                # issuing them late to hide in the taper tail measured worse.)
                vals = gp.tile([P, NB], i32, tag="vals")
                nc.gpsimd.dma_start(out=vals[:], in_=value_pb)
                off = gp.tile([P, NB], i32, tag="off")
                nc.gpsimd.tensor_tensor(out=off[:], in0=ioff[:], in1=vals[:], op=add)
                xvs = gp.tile([P, NB], f32, tag="xvs")
                for gb in range(NB):
                    nc.gpsimd.indirect_dma_start(
                        out=xvs[:, gb : gb + 1],
                        out_offset=None,
                        in_=logits_flat,
                        in_offset=bass.IndirectOffsetOnAxis(
                            ap=off[:, gb : gb + 1], axis=0
                        ),
                    )
                # per-block row sums, finalized once in the epilogue
                Zall = gp.tile([P, NB], f32, tag="Zall")
                Sall = gp.tile([P, NB], f32, tag="Sall")

                for rb in range(NB):
                    rows = slice(rb * P, (rb + 1) * P)
                    # Taper the very last chunks of the run so the trailing
                    # ACT+DVE work after the final DMA lands is short.
                    if rb == NB - 1 and rep == reps - 1:
                        widths = [CC] * (NCH - 2) + [
                            CC // 2, CC // 2, CC // 2, CC // 4, CC // 8, CC // 8,
                        ]
                    else:
                        widths = [CC] * NCH
                    nch = len(widths)
                    zparts = sp.tile([P, nch], f32, tag="zparts")
                    sparts = sp.tile([P, nch], f32, tag="sparts")
                    c0 = 0
                    for ch, w in enumerate(widths):
                        cols = slice(c0, c0 + w)
                        c0 += w
                        x = xp.tile([P, w], f32, tag="x")
                        nc.sync.dma_start(out=x[:], in_=logits[rows, cols])
                        u = up.tile([P, w], f32, tag="u")
                        nc.scalar.activation(
                            u[:], x[:], Exp, accum_out=zparts[:, ch : ch + 1]
                        )
                        # fused multiply+reduce; stride-0 dummy out discards
                        # the product, only sparts[:, ch] (the sum) is kept
                        dummy = sp.tile([P, 1], f32, tag="dummy")
                        nc.vector._custom_dve(
                            TENSOR_TENSOR_REDUCE,
                            out=dummy.broadcast_to(u[:].shape),
                            in0=u[:],
                            in1=x[:],
                            s0=0.0,
                            s1=1.0,
                            accum_out=sparts[:, ch : ch + 1],
                        )

                    # --- per-block partial reduction (DVE only, no ACT) ---
                    nc.vector.tensor_reduce(
                        Zall[:, rb : rb + 1], zparts[:], axis=X, op=add
                    )
                    nc.vector.tensor_reduce(
                        Sall[:, rb : rb + 1], sparts[:], axis=X, op=add
                    )

                # --- final epilogue, all blocks at once ([P, NB] ops) ---
                res = gp.tile([P, NB * 3], f32, tag="res")
                res3 = res[:].rearrange("p (b k) -> p b k", b=NB)
                rZ = gp.tile([P, NB], f32, tag="rZ")
                nc.vector.reciprocal(rZ[:], Zall[:])
                logZ = gp.tile([P, NB], f32, tag="logZ")
                nc.scalar.activation(logZ[:], Zall[:], Ln)
                # log_prob = x_v - log Z
                nc.vector.tensor_sub(out=res3[:, :, 1], in0=xvs[:], in1=logZ[:])
                # pdf = exp(log_prob) (ACT, combined table -> no table switch)
                nc.scalar.activation(res3[:, :, 0], res3[:, :, 1], Exp)
                # entropy = S/Z - log Z
                sz = gp.tile([P, NB], f32, tag="sz")
                nc.vector.tensor_mul(out=sz[:], in0=Sall[:], in1=rZ[:])
                nc.vector.tensor_sub(out=res3[:, :, 2], in0=sz[:], in1=logZ[:])
                # issue the store from ACT's HWDGE ring: an SP-issued store
                # would wait here for the epilogue and head-of-line block the
                # next rep's chunk loads queued behind it on SP
                nc.scalar.dma_start(out=out_pb, in_=res3)

    # Pre-place a single load of the combined Exp+Ln activation table at
    # program start; the compile-time fixpoint then sees every activation's
    # table already loaded and inserts no further loads (the default
    # placement reloads Exp-table <-> Ln-table every iteration).
    tabs = list(get_activation_tables(nc.m.arch).values())
    combined = next(i for i, s in enumerate(tabs) if Exp in s and Ln in s)
    load = mybir.InstLoadActFuncSet(
        name=nc.get_next_instruction_name(), ins=[], outs=[]
    )
    load.engine = mybir.EngineType.Activation
    load.act_func_set_id = combined
    nc.register_instruction(load)
    nc.main_func.blocks[0].instructions.insert(0, load)

    nc.compile()
    return nc


def kernel(logits, value):
    global LAST_RESULT
    logits = np.ascontiguousarray(np.asarray(logits), dtype=np.float32)
    value = np.asarray(value).astype(np.int32).reshape(B, 1)
    assert logits.shape == (B, V)

    if "nc" not in _CACHE:
        _CACHE["nc"] = _build_bass()
    nc = _CACHE["nc"]

    in_maps = [
        {
            "logits": np.ascontiguousarray(logits[c * R : (c + 1) * R]),
            "value": np.ascontiguousarray(value[c * R : (c + 1) * R]),
        }
        for c in range(NCORES)
    ]
    result = run_bass_kernel_spmd(
        nc, in_maps, core_ids=list(range(NCORES)), trace=TRACE
    )
    LAST_RESULT = result
    # each core's out is [R, 3]; full output is [3, B]
    full = np.concatenate([r["out"] for r in result.results], axis=0)  # [B, 3]
    return np.ascontiguousarray(full.T)
